# revision 20
# baseline (speedup 1.0000x reference)
"""Trainium2 Bass kernel for nn_AutoregressiveInferenceNet (moe_routing).

Sharding: data-parallel over batch (8 samples/core, zero collectives).
Expert tables are dispatched per-core by node id on the host (index/routing
metadata only); all FP compute and large data movement happens on device.

kernel(**inputs) -> (outs [T-1,B,D], alphas [T-1,B,T-1,HEADS])
"""

import numpy as np
from contextlib import ExitStack

# ---- problem dims (hardcoded per contract) ----
B, T, N, E, ENC, HID, HEADS, D = 64, 128, 2000, 300, 256, 256, 5, 16
IN = ENC + 2 * E            # 856
ATT = HID * HEADS           # 1280
NCORES = 8
BC = B // NCORES            # 8 samples per core
NCHAIN = 2                  # independent GRU chains per core
BCH = BC // NCHAIN          # 4 samples per chain

# x^T K-tile structure: [label 300 | program 256 | node 300] = 856
# staging columns (padded sections to 128 multiples for clean transposes):
# label -> cols 0:300, program -> 384:640, node -> 640:940, staging width 1024
KTILE_SRC_COL = [0, 128, 256, 384, 512, 640, 768, 896]   # staging col of tile
KTILE_ROWS = [128, 128, 44, 128, 128, 128, 128, 44]      # valid K per tile
# row offset of each K-tile inside wihT [856, 768]
KTILE_WROW = [0, 128, 256, 300, 428, 556, 684, 812]

SLOT_S = 2                  # pairs per expert-group slot block
MPACK = 8                   # expert groups packed per matmul (M = 8*16 = 128)


def _build_program(Tm1, G_PAD, mybir, bass, bacc, tile):
    """Build the SPMD Bass program (same program for all 8 cores)."""
    dt = mybir.dt
    f32 = dt.float32
    bf16 = dt.float16  # fp16: 1cyc/row like bf16, 8x mantissa
    i32 = dt.int32
    AF = mybir.ActivationFunctionType
    OP = mybir.AluOpType
    from concourse.masks import make_identity
    from concourse.bass import IndirectOffsetOnAxis

    PAIRS = Tm1 * BC
    NCHUNK = PAIRS // 127 if PAIRS % 127 == 0 else None
    # gather chunking: chunks of <=127 pairs
    chunk_bounds = list(range(0, PAIRS, 127)) + [PAIRS]
    SLOTS = SLOT_S * G_PAD
    NPACK = G_PAD // MPACK

    nc = bacc.Bacc("TRN2", target_bir_lowering=False, debug=False,
                   num_devices=NCORES)

    # ---- DRAM tensors (inputs) ----
    d_label = nc.dram_tensor("label_flat", [(N + 1) * D, E], f32, kind="ExternalInput")
    d_node = nc.dram_tensor("node_emb", [N + 1, E], f32, kind="ExternalInput")
    d_prog = nc.dram_tensor("prog_rows", [BC, ENC], f32, kind="ExternalInput")
    d_h0T = nc.dram_tensor("h0T", [HID, BC], f32, kind="ExternalInput")
    d_wihT = nc.dram_tensor("wihT", [IN, 3 * HID], f32, kind="ExternalInput")
    d_whhT = nc.dram_tensor("whhT", [HID, 3 * HID], f32, kind="ExternalInput")
    d_biasc = nc.dram_tensor("bias_comb", [128, 6], f32, kind="ExternalInput")
    d_bhhn = nc.dram_tensor("bhh_n", [128, 2 * BC], f32, kind="ExternalInput")
    d_ws1T = nc.dram_tensor("ws1T", [HID, HID], f32, kind="ExternalInput")
    d_ws2T = nc.dram_tensor("ws2T", [HID, HEADS], f32, kind="ExternalInput")
    d_triT = nc.dram_tensor("triT", [Tm1, Tm1], f32, kind="ExternalInput")
    d_lidx = nc.dram_tensor("label_idx", [PAIRS, 1], i32, kind="ExternalInput")
    d_nidx = nc.dram_tensor("node_idx", [PAIRS, 1], i32, kind="ExternalInput")
    d_bofp = nc.dram_tensor("b_of_pair", [PAIRS, 1], i32, kind="ExternalInput")
    d_slot = nc.dram_tensor("slot_idx", [Tm1, BC], i32, kind="ExternalInput")
    d_expw = nc.dram_tensor("exp_wt", [NPACK, 128, 10, 128], bf16, kind="ExternalInput")
    d_biasT = nc.dram_tensor("slot_biasT", [128, SLOTS], f32,
                             kind="ExternalInput")

    # ---- DRAM internal + outputs ----
    d_ctxs = nc.dram_tensor("ctx_sorted", [SLOTS, ATT], bf16)
    d_alphas = nc.dram_tensor("alphas_core", [Tm1, BC, Tm1, HEADS], f32,
                              kind="ExternalOutput")
    d_predpk = nc.dram_tensor("pred_packed", [128, SLOTS], f32,
                              kind="ExternalOutput")

    with tile.TileContext(nc) as tc, ExitStack() as ctx:
        # ================= persistent pools =================
        const_p = ctx.enter_context(tc.tile_pool(name="const", bufs=1))
        ident = const_p.tile([128, 128], f32)
        make_identity(nc, ident[:])
        triT_sb = const_p.tile([Tm1, Tm1], f32)
        nc.sync.dma_start(triT_sb[:], d_triT.ap())
        whhT_sb = const_p.tile([128, 2, 3 * HID], f32)
        for k in range(2):
            nc.sync.dma_start(whhT_sb[:, k, :], d_whhT.ap()[k * 128:(k + 1) * 128, :])
        biasc_sb = const_p.tile([128, 6], f32)
        nc.sync.dma_start(biasc_sb[:], d_biasc.ap())
        bhhn_sb = const_p.tile([128, 2 * BC], f32)
        nc.sync.dma_start(bhhn_sb[:], d_bhhn.ap())
        ws1T_sb = const_p.tile([128, 2, HID], f32)
        for k in range(2):
            nc.sync.dma_start(ws1T_sb[:, k, :], d_ws1T.ap()[k * 128:(k + 1) * 128, :])
        ws2T_sb = const_p.tile([128, 2, HEADS], f32)
        for k in range(2):
            nc.sync.dma_start(ws2T_sb[:, k, :], d_ws2T.ap()[k * 128:(k + 1) * 128, :])

        big_p = ctx.enter_context(tc.tile_pool(name="big", bufs=1))
        # gi: free layout t*48 + m*8 + b
        gi_sb = big_p.tile([128, Tm1 * 6 * BC], f32)
        # H stores per chain: [128, kappa, t-slot(Tm1+1), bch]
        Hst = [big_p.tile([128, 2, Tm1 + 1, BCH], f32, name=f"Hst{c}")
               for c in range(NCHAIN)]

        # h0 into slot 0
        for c in range(NCHAIN):
            for k in range(2):
                nc.sync.dma_start(
                    Hst[c][:, k, 0, :],
                    d_h0T.ap()[k * 128:(k + 1) * 128, c * BCH:(c + 1) * BCH])

        # ================= phase 1: gather + transpose + gi GEMM ==========
        with tc.tile_pool(name="p1w", bufs=1) as p1w, \
             tc.tile_pool(name="p1", bufs=2) as p1, \
             tc.tile_pool(name="p1ps", bufs=2, space="PSUM") as p1ps, \
             tc.tile_pool(name="xt", bufs=1) as xtp:
            # Wih^T K-tiles
            wih_k = []
            for kt in range(8):
                w = p1w.tile([128, 3 * HID], f32, name=f"wih{kt}")
                nc.sync.dma_start(
                    w[0:KTILE_ROWS[kt], :],
                    d_wihT.ap()[KTILE_WROW[kt]:KTILE_WROW[kt] + KTILE_ROWS[kt], :])
                wih_k.append(w)
            # x^T K-tiles
            xT = [xtp.tile([128, PAIRS], f32, name=f"xT{kt}") for kt in range(8)]

            for ci in range(len(chunk_bounds) - 1):
                p0, p1e = chunk_bounds[ci], chunk_bounds[ci + 1]
                np_c = p1e - p0
                stg = p1.tile([128, 1024], f32, tag="stg")
                lidx = p1.tile([128, 1], i32, tag="lidx")
                nidx = p1.tile([128, 1], i32, tag="nidx")
                bidx = p1.tile([128, 1], i32, tag="bidx")
                nc.sync.dma_start(lidx[0:np_c, :], d_lidx.ap()[p0:p1e, :])
                nc.sync.dma_start(nidx[0:np_c, :], d_nidx.ap()[p0:p1e, :])
                nc.sync.dma_start(bidx[0:np_c, :], d_bofp.ap()[p0:p1e, :])
                nc.gpsimd.indirect_dma_start(
                    out=stg[0:np_c, 0:E], out_offset=None,
                    in_=d_label.ap(),
                    in_offset=IndirectOffsetOnAxis(ap=lidx[0:np_c, 0:1], axis=0))
                nc.gpsimd.indirect_dma_start(
                    out=stg[0:np_c, 384:384 + ENC], out_offset=None,
                    in_=d_prog.ap(),
                    in_offset=IndirectOffsetOnAxis(ap=bidx[0:np_c, 0:1], axis=0))
                nc.gpsimd.indirect_dma_start(
                    out=stg[0:np_c, 640:640 + E], out_offset=None,
                    in_=d_node.ap(),
                    in_offset=IndirectOffsetOnAxis(ap=nidx[0:np_c, 0:1], axis=0))
                for kt in range(8):
                    pt = p1ps.tile([128, 128], f32, tag="tp", space="PSUM")
                    c0 = KTILE_SRC_COL[kt]
                    nc.tensor.transpose(pt[0:128, 0:np_c],
                                        stg[0:np_c, c0:c0 + 128],
                                        ident[0:np_c, 0:np_c])
                    kr = KTILE_ROWS[kt]
                    nc.vector.tensor_copy(xT[kt][0:kr, p0:p1e], pt[0:kr, 0:np_c])

            # gi GEMM: out gi^T [768, PAIRS] in m-tiles
            nhalf = (PAIRS // 2) // BC * BC  # multiple of BC
            for (n0, n1) in [(0, nhalf), (nhalf, PAIRS)]:
                nsz = n1 - n0
                for m in range(6):
                    ps = p1ps.tile([128, 512], f32, tag="gemm", space="PSUM")
                    for kt in range(8):
                        kr = KTILE_ROWS[kt]
                        nc.tensor.matmul(
                            ps[:, 0:nsz],
                            lhsT=wih_k[kt][0:kr, m * 128:(m + 1) * 128],
                            rhs=xT[kt][0:kr, n0:n1],
                            start=(kt == 0), stop=(kt == 7))
                    # bias add + scatter into gi layout [t*48 + m*8 + b]
                    t0 = n0 // BC
                    nt = nsz // BC
                    gi_view = gi_sb[:].rearrange(
                        "p (t x) -> p t x", x=6 * BC)[:, t0:t0 + nt,
                                                      m * BC:(m + 1) * BC]
                    nc.vector.tensor_scalar_add(
                        gi_view, ps[:].rearrange("p (t b) -> p t b", b=BC)[:, 0:nt, :],
                        biasc_sb[:, m:m + 1])

        # ================= phase 2: GRU loop =================
        with tc.tile_pool(name="p2ps", bufs=2, space="PSUM") as p2ps, \
             tc.tile_pool(name="p2", bufs=2) as p2:
            gi_v = gi_sb[:].rearrange("p (t m b) -> p t m b", m=6, b=BC)
            for t in range(Tm1):
                for c in range(NCHAIN):
                    b0 = c * BCH
                    ps_rz = p2ps.tile([128, 4 * BCH], f32, tag=f"rz{c}", space="PSUM")
                    ps_hn = p2ps.tile([128, 2 * BCH], f32, tag=f"hn{c}", space="PSUM")
                    # preload gi (r,z part) and b_hh (n part) into psum via
                    # identity matmuls (exact; off the h-dependency path)
                    nc.tensor.matmul(
                        ps_rz[:].rearrange("p (m b) -> p m b", b=BCH),
                        lhsT=ident[:], rhs=gi_v[:, t, 0:4, b0:b0 + BCH],
                        start=True, stop=False)
                    nc.tensor.matmul(
                        ps_hn[:].rearrange("p (k b) -> p k b", b=BCH),
                        lhsT=ident[:],
                        rhs=bhhn_sb[:].rearrange("p (k b) -> p k b",
                                                 b=BC)[:, :, b0:b0 + BCH],
                        start=True, stop=False)
                    h_prev = Hst[c][:, :, t, :]
                    for m in range(6):
                        out_ap = (ps_rz[:, m * BCH:(m + 1) * BCH] if m < 4
                                  else ps_hn[:, (m - 4) * BCH:(m - 3) * BCH])
                        for k in range(2):
                            last = (k == 1) and (m == 3 or m == 5)
                            nc.tensor.matmul(
                                out_ap,
                                lhsT=whhT_sb[:, k, m * 128:(m + 1) * 128],
                                rhs=Hst[c][:, k, t, :],
                                start=False, stop=last)
                    rz = p2.tile([128, 4 * BCH], f32, tag=f"sig{c}")
                    nc.scalar.activation(rz[:], ps_rz[:], AF.Sigmoid)
                    rhn = p2.tile([128, 2 * BCH], f32, tag=f"rhn{c}")
                    nc.vector.tensor_tensor(rhn[:], rz[:, 0:2 * BCH], ps_hn[:],
                                            op=OP.mult)
                    npre = p2.tile([128, 2 * BCH], f32, tag=f"npre{c}")
                    nc.vector.tensor_tensor(
                        npre[:].rearrange("p (m b) -> p m b", b=BCH),
                        rhn[:].rearrange("p (m b) -> p m b", b=BCH),
                        gi_v[:, t, 4:6, b0:b0 + BCH], op=OP.add)
                    nt_ = p2.tile([128, 2 * BCH], f32, tag=f"tanh{c}")
                    nc.scalar.activation(nt_[:], npre[:], AF.Tanh)
                    hmn = p2.tile([128, 2 * BCH], f32, tag=f"hmn{c}")
                    nc.vector.tensor_tensor(
                        hmn[:].rearrange("p (k b) -> p k b", b=BCH),
                        h_prev,
                        nt_[:].rearrange("p (k b) -> p k b", b=BCH),
                        op=OP.subtract)
                    zm = p2.tile([128, 2 * BCH], f32, tag=f"zm{c}")
                    nc.vector.tensor_tensor(zm[:], rz[:, 2 * BCH:4 * BCH], hmn[:],
                                            op=OP.mult)
                    nc.vector.tensor_tensor(
                        Hst[c][:, :, t + 1, :],
                        nt_[:].rearrange("p (k b) -> p k b", b=BCH),
                        zm[:].rearrange("p (k b) -> p k b", b=BCH),
                        op=OP.add)

        # ================= phase 3a: scores / E =================
        p3a = ctx.enter_context(tc.tile_pool(name="p3a", bufs=1))
        E_T = []  # per chain [HEADS, Tm1*BCH]
        with tc.tile_pool(name="p3aps", bufs=2, space="PSUM") as psp:
            for c in range(NCHAIN):
                ncols = Tm1 * BCH
                a1 = p3a.tile([128, 2, ncols], f32, name=f"a1_{c}")
                for m in range(2):
                    ps = psp.tile([128, 512], f32, tag="a1ps", space="PSUM")
                    for k in range(2):
                        nc.tensor.matmul(
                            ps[:, 0:ncols],
                            lhsT=ws1T_sb[:, k, m * 128:(m + 1) * 128],
                            rhs=Hst[c][:, k, 1:Tm1 + 1, :],
                            start=(k == 0), stop=(k == 1))
                    nc.scalar.activation(a1[:, m, :], ps[:, 0:ncols], AF.Tanh)
                et = p3a.tile([HEADS, ncols], f32, name=f"eT{c}")
                ps2 = psp.tile([HEADS, 512], f32, tag="sps", space="PSUM")
                for k in range(2):
                    nc.tensor.matmul(ps2[:, 0:ncols],
                                     lhsT=ws2T_sb[:, k, :],
                                     rhs=a1[:, k, :],
                                     start=(k == 0), stop=(k == 1))
                nc.scalar.activation(et[:], ps2[:, 0:ncols], AF.Exp)
                E_T.append(et)

        # ================= phase 3b: per-sample attention =================
        with tc.tile_pool(name="p3b", bufs=2) as p3b, \
             tc.tile_pool(name="p3bps", bufs=2, space="PSUM") as p3bps, \
             tc.tile_pool(name="p3bo", bufs=2) as p3bo:
            for bl in range(BC):
                c, off = bl // BCH, bl % BCH
                # E_b [Tm1, HEADS]
                psE = p3bps.tile([Tm1, HEADS], f32, tag="psE", space="PSUM")
                eT_b = E_T[c][:].rearrange("h (t b) -> h t b", b=BCH)[:, :, off]
                nc.tensor.transpose(psE[:], eT_b, ident[0:HEADS, 0:HEADS])
                E_b = p3b.tile([Tm1, HEADS], f32, tag="Eb")
                nc.vector.tensor_copy(E_b[:], psE[:])
                # Haug [Tm1, 257]
                Haug = p3b.tile([Tm1, 2 * 128 + 1], f32, tag="Haug")
                for k in range(2):
                    psT = p3bps.tile([Tm1, 128], f32, tag="psT", space="PSUM")
                    nc.tensor.transpose(psT[:], Hst[c][:, k, 1:Tm1 + 1, off],
                                        ident[:])
                    nc.vector.tensor_copy(Haug[:, k * 128:(k + 1) * 128], psT[:])
                nc.gpsimd.memset(Haug[:, 256:257], 1.0)

                ctxb = p3bo.tile([Tm1, ATT], bf16, tag="ctxb")
                alph = p3bo.tile([Tm1, Tm1 * HEADS], f32, tag="alph")
                for h in range(HEADS):
                    At = p3b.tile([Tm1, Tm1], f32, tag="At")
                    nc.vector.tensor_scalar_mul(At[:], triT_sb[:], E_b[:, h:h + 1])
                    psN = p3bps.tile([Tm1, 257], f32, tag="psN", space="PSUM")
                    nc.tensor.matmul(psN[:], lhsT=At[:], rhs=Haug[:],
                                     start=True, stop=True)
                    rZ = p3b.tile([Tm1, 1], f32, tag="rZ")
                    nc.vector.reciprocal(rZ[:], psN[:, 256:257])
                    nc.vector.tensor_scalar_mul(
                        ctxb[:, h * HID:(h + 1) * HID], psN[:, 0:HID], rZ[:])
                    psA = p3bps.tile([Tm1, Tm1], f32, tag="psA", space="PSUM")
                    nc.tensor.transpose(psA[:], At[:], ident[0:Tm1, 0:Tm1])
                    alph_v = alph[:].rearrange("t (j h) -> t j h", h=HEADS)
                    nc.vector.tensor_scalar_mul(alph_v[:, :, h], psA[:], rZ[:])
                # write alphas rows [t, bl, :, :]
                nc.sync.dma_start(
                    d_alphas.ap()[:, bl, :, :].rearrange("t j h -> t (j h)"),
                    alph[:])
                # scatter ctx rows to slots
                sidx = p3b.tile([Tm1, 1], i32, tag="sidx")
                nc.sync.dma_start(sidx[:], d_slot.ap()[:, bl:bl + 1])
                nc.gpsimd.indirect_dma_start(
                    out=d_ctxs.ap(), out_offset=IndirectOffsetOnAxis(
                        ap=sidx[:, 0:1], axis=0),
                    in_=ctxb[:], in_offset=None)

        # ================= phase 3c: pred head =================
        with tc.tile_pool(name="p3c", bufs=4) as p3c, \
             tc.tile_pool(name="p3cw", bufs=8) as p3cw, \
             tc.tile_pool(name="p3cps", bufs=1, space="PSUM") as p3cps, \
             tc.tile_pool(name="p3co", bufs=1) as p3co:
            ctxT = p3co.tile([128, 10, SLOTS], bf16)
            for k in range(10):
                nc.sync.dma_start_transpose(
                    ctxT[:, k, :], d_ctxs.ap()[:, k * 128:(k + 1) * 128])
            ps_pred = p3cps.tile([128, 16 * NPACK], f32, space="PSUM")
            for pk in range(NPACK):
                w = p3cw.tile([128, 10, 128], bf16, tag="wpk")
                nc.sync.dma_start(w[:], d_expw.ap()[pk])
                for k in range(10):
                    nc.tensor.matmul(
                        ps_pred[:, pk * 16:(pk + 1) * 16],
                        lhsT=w[:, k, :],
                        rhs=ctxT[:, k, pk * 16:(pk + 1) * 16],
                        start=(k == 0), stop=(k == 9))
            # bias add in packed layout; host unshard extracts slots
            biasP_sb = p3co.tile([128, 16 * NPACK], f32)
            nc.sync.dma_start(biasP_sb[:], d_biasT.ap())
            predall = p3co.tile([128, 16 * NPACK], f32)
            nc.vector.tensor_tensor(predall[:], ps_pred[:], biasP_sb[:],
                                    op=OP.add)
            nc.sync.dma_start(d_predpk.ap(), predall[:])

    nc.compile()
    return nc


# ======================= host staging =======================

def _stage(inputs, Tm1):
    """Compute per-core input maps + unshard metadata (host: slicing,
    transposes, dtype casts, and integer routing metadata only)."""
    pe = np.ascontiguousarray(inputs["program_emb"], dtype=np.float32)
    h0 = np.ascontiguousarray(inputs["h0"], dtype=np.float32)
    node_emb = np.ascontiguousarray(inputs["node_emb"], dtype=np.float32)
    label_flat = np.ascontiguousarray(
        inputs["label_emb"], dtype=np.float32).reshape((N + 1) * D, E)
    Wih = np.asarray(inputs["gru_w_ih"], dtype=np.float32)
    Whh = np.asarray(inputs["gru_w_hh"], dtype=np.float32)
    bih = np.asarray(inputs["gru_b_ih"], dtype=np.float32)
    bhh = np.asarray(inputs["gru_b_hh"], dtype=np.float32)
    ws1 = np.asarray(inputs["attn_ws1"], dtype=np.float32)
    ws2 = np.asarray(inputs["attn_ws2"], dtype=np.float32)
    pred_w = np.asarray(inputs["pred_w"], dtype=np.float32)
    pred_b = np.asarray(inputs["pred_b"], dtype=np.float32)
    trace = np.asarray(inputs["execution_trace"])
    na = np.asarray(inputs["node_assignments"])

    PAIRS = Tm1 * BC
    na_pad = np.concatenate([na, np.zeros((B, 1), na.dtype)], axis=1)

    bias_comb = np.concatenate([(bih + bhh)[:512], bih[512:768]])
    wihT = np.ascontiguousarray(Wih.T)
    whhT = np.ascontiguousarray(Whh.T)
    ws1T = np.ascontiguousarray(ws1.T)
    ws2T = np.ascontiguousarray(ws2.T)
    triT = (np.arange(Tm1)[None, :] >= np.arange(Tm1)[:, None]).astype(np.float32)
    triT = np.ascontiguousarray(triT)
    bias_comb_sb = np.ascontiguousarray(bias_comb.reshape(6, 128).T)
    bhh_n = np.ascontiguousarray(
        np.repeat(bhh[512:768].reshape(2, 128).T[:, :, None], BC, axis=2)
        .reshape(128, 2 * BC))

    # per-core routing
    per_core = []
    for c in range(NCORES):
        bs = np.arange(c * BC, (c + 1) * BC)
        tt = np.arange(Tm1)
        # pair p = t*BC + bl
        node_i = trace[bs][:, :Tm1].T            # [Tm1, BC]
        lab = (node_i * D + na_pad[bs[None, :], node_i]).astype(np.int32)
        experts = trace[bs][:, 1:Tm1 + 1].T.astype(np.int64)   # [Tm1, BC]
        per_core.append(dict(
            label_idx=lab.reshape(-1).astype(np.int32),
            node_idx=node_i.reshape(-1).astype(np.int32),
            experts=experts.reshape(-1),
        ))

    # expert grouping (fixed S=2 slots per group)
    G_max = 0
    for pc in per_core:
        exp = pc["experts"]
        order = np.lexsort((np.arange(PAIRS), exp))
        groups = []          # list of (expert, [pair indices])
        cur_e, cur = None, []
        for p in order:
            e = exp[p]
            if e == cur_e and len(cur) < SLOT_S:
                cur.append(p)
            else:
                if cur:
                    groups.append((cur_e, cur))
                cur_e, cur = e, [p]
        if cur:
            groups.append((cur_e, cur))
        pc["groups"] = groups
        G_max = max(G_max, len(groups))
    G_PAD = -(-G_max // (MPACK * 2)) * (MPACK * 2)   # mult of 16
    SLOTS = SLOT_S * G_PAD

    in_maps, metas = [], []
    for c in range(NCORES):
        pc = per_core[c]
        groups = pc["groups"]
        slot_idx = np.zeros((Tm1, BC), np.int32)
        slot_pair = np.full(SLOTS, -1, np.int64)
        gexp = np.zeros(G_PAD, np.int64)
        for g, (e, plist) in enumerate(groups):
            gexp[g] = e
            for si, p in enumerate(plist):
                s = g * SLOT_S + si
                slot_pair[s] = p
                slot_idx[p // BC, p % BC] = s
        # M-packed weight tiles [NPACK, 128(r), 10(k), 128(m)]
        NPACK = G_PAD // MPACK
        wsel = pred_w[gexp]                      # [G_PAD, 1280, 16]
        wt = wsel.reshape(NPACK, MPACK, 10, 128, D)      # [pk, g, k, r, j]
        wt = wt.transpose(0, 3, 2, 1, 4)                 # [pk, r, k, g, j]
        exp_wt = np.ascontiguousarray(
            wt.reshape(NPACK, 128, 10, 128)).astype(np.float16)
        # packed-layout bias: rows 16g+k, col pk*16 + g*2 + s
        bsel = pred_b[gexp]                      # [G_PAD, 16]
        slot_biasT = np.zeros((128, 16 * NPACK), np.float32)
        for g_abs in range(G_PAD):
            pk, g = g_abs // MPACK, g_abs % MPACK
            for s in range(SLOT_S):
                slot_biasT[16 * g:16 * (g + 1),
                           pk * 16 + g * SLOT_S + s] = bsel[g_abs]

        bs = slice(c * BC, (c + 1) * BC)
        in_maps.append({
            "label_flat": label_flat,
            "node_emb": node_emb,
            "prog_rows": np.ascontiguousarray(pe[bs]),
            "h0T": np.ascontiguousarray(h0[bs].T),
            "wihT": wihT, "whhT": whhT,
            "bias_comb": bias_comb_sb, "bhh_n": bhh_n,
            "ws1T": ws1T, "ws2T": ws2T, "triT": triT,
            "label_idx": pc["label_idx"].reshape(-1, 1),
            "node_idx": pc["node_idx"].reshape(-1, 1),
            "b_of_pair": (np.arange(PAIRS) % BC).astype(np.int32).reshape(-1, 1),
            "slot_idx": slot_idx,
            "exp_wt": exp_wt,
            "slot_biasT": slot_biasT,
        })
        metas.append(dict(slot_pair=slot_pair))
    return in_maps, metas, G_PAD


def _unshard(results, metas, Tm1):
    outs = np.zeros((Tm1, B, D), np.float32)
    alphas = np.zeros((Tm1, B, Tm1, HEADS), np.float32)
    for c in range(NCORES):
        r = results[c]
        alphas[:, c * BC:(c + 1) * BC] = r["alphas_core"]
        slot_pair = metas[c]["slot_pair"]
        pp = r["pred_packed"]
        slots = np.nonzero(slot_pair >= 0)[0]
        ps = slot_pair[slots]
        pk, rem = slots // (MPACK * SLOT_S), slots % (MPACK * SLOT_S)
        g, s = rem // SLOT_S, rem % SLOT_S
        cols = pk * 16 + g * SLOT_S + s
        rows = (g[:, None] * 16 + np.arange(D)[None, :])
        vals = pp[rows, cols[:, None]]           # [n_valid, D]
        outs[ps // BC, c * BC + (ps % BC)] = vals
    return outs, alphas


_PROG_CACHE = {}
LAST_EXEC_NS = None


def kernel(**inputs):
    import concourse.bass as bass
    import concourse.mybir as mybir
    import concourse.tile as tile
    from concourse import bacc
    from concourse.bass_utils import run_bass_kernel_spmd

    Tm1 = T - 1
    in_maps, metas, G_PAD = _stage(inputs, Tm1)
    key = (Tm1, G_PAD)
    if key not in _PROG_CACHE:
        _PROG_CACHE[key] = _build_program(Tm1, G_PAD, mybir, bass, bacc, tile)
    nc = _PROG_CACHE[key]
    res = run_bass_kernel_spmd(nc, in_maps, list(range(NCORES)))
    global LAST_EXEC_NS
    LAST_EXEC_NS = res.exec_time_ns
    return _unshard(res.results, metas, Tm1)


# revision 25
# speedup vs baseline: 2.2170x; 2.2170x over previous
"""Trainium2 Bass kernel for nn_AutoregressiveInferenceNet (moe_routing).

Sharding: data-parallel over batch (8 samples/core, zero collectives).
Expert tables are dispatched per-core by node id on the host (index/routing
metadata only); all FP compute and large data movement happens on device.

kernel(**inputs) -> (outs [T-1,B,D], alphas [T-1,B,T-1,HEADS])
"""

import numpy as np
from contextlib import ExitStack

# ---- problem dims (hardcoded per contract) ----
B, T, N, E, ENC, HID, HEADS, D = 64, 128, 2000, 300, 256, 256, 5, 16
IN = ENC + 2 * E            # 856
ATT = HID * HEADS           # 1280
NCORES = 8
BC = B // NCORES            # 8 samples per core
NCHAIN = 2                  # independent GRU chains per core
BCH = BC // NCHAIN          # 4 samples per chain

# x^T K-tile structure: [label 300 | program 256 | node 300] = 856
# staging columns (padded sections to 128 multiples for clean transposes):
# label -> cols 0:300, program -> 384:640, node -> 640:940, staging width 1024
KTILE_SRC_COL = [0, 128, 256, 384, 512, 640, 768, 896]   # staging col of tile
KTILE_ROWS = [128, 128, 44, 128, 128, 128, 128, 44]      # valid K per tile
# row offset of each K-tile inside wihT [856, 768]
KTILE_WROW = [0, 128, 256, 300, 428, 556, 684, 812]

SLOT_S = 2                  # pairs per expert-group slot block
MPACK = 8                   # expert groups packed per matmul (M = 8*16 = 128)


def _build_program(Tm1, G_PAD, mybir, bass, bacc, tile):
    """Build the SPMD Bass program (same program for all 8 cores)."""
    dt = mybir.dt
    f32 = dt.float32
    bf16 = dt.float16  # fp16: 1cyc/row like bf16, 8x mantissa
    i32 = dt.int32
    AF = mybir.ActivationFunctionType
    OP = mybir.AluOpType
    from concourse.masks import make_identity
    from concourse.bass import IndirectOffsetOnAxis

    PAIRS = Tm1 * BC
    NCHUNK = PAIRS // 127 if PAIRS % 127 == 0 else None
    # gather chunking: chunks of <=127 pairs
    chunk_bounds = list(range(0, PAIRS, 127)) + [PAIRS]
    SLOTS = SLOT_S * G_PAD
    NPACK = G_PAD // MPACK

    nc = bacc.Bacc("TRN2", target_bir_lowering=False, debug=False,
                   num_devices=NCORES)

    # ---- DRAM tensors (inputs) ----
    d_label = nc.dram_tensor("label_flat", [(N + 1) * D, E], f32, kind="ExternalInput")
    d_node = nc.dram_tensor("node_emb", [N + 1, E], f32, kind="ExternalInput")
    d_prog = nc.dram_tensor("prog_rows", [BC, ENC], f32, kind="ExternalInput")
    d_h0T = nc.dram_tensor("h0T", [HID, BC], f32, kind="ExternalInput")
    d_wihT = nc.dram_tensor("wihT", [IN, 3 * HID], f32, kind="ExternalInput")
    d_whh16 = nc.dram_tensor("whhT16", [HID, 3 * HID], bf16, kind="ExternalInput")
    d_h016 = nc.dram_tensor("h0T16", [HID, BC], bf16, kind="ExternalInput")
    d_biasc = nc.dram_tensor("bias_comb", [128, 6], f32, kind="ExternalInput")
    d_bhhn = nc.dram_tensor("bhh_n2", [128, 2], f32, kind="ExternalInput")
    d_ws1T = nc.dram_tensor("ws1T", [HID, HID], f32, kind="ExternalInput")
    d_ws2T = nc.dram_tensor("ws2T", [HID, HEADS], f32, kind="ExternalInput")
    d_triT = nc.dram_tensor("triT", [Tm1, Tm1], f32, kind="ExternalInput")
    d_lidx = nc.dram_tensor("label_idx", [PAIRS, 1], i32, kind="ExternalInput")
    d_nidx = nc.dram_tensor("node_idx", [PAIRS, 1], i32, kind="ExternalInput")
    d_bofp = nc.dram_tensor("b_of_pair", [PAIRS, 1], i32, kind="ExternalInput")
    d_slot = nc.dram_tensor("slot_idx", [Tm1, BC], i32, kind="ExternalInput")
    d_expw = nc.dram_tensor("exp_wt", [NPACK, 128, 10, 128], bf16, kind="ExternalInput")
    d_biasT = nc.dram_tensor("slot_biasT", [128, SLOTS], f32,
                             kind="ExternalInput")

    # ---- DRAM internal + outputs ----
    d_ctxs = nc.dram_tensor("ctx_sorted", [SLOTS, ATT], bf16)
    d_alphas = nc.dram_tensor("alphas_core", [Tm1, BC, Tm1, HEADS], f32,
                              kind="ExternalOutput")
    d_predpk = nc.dram_tensor("pred_packed", [128, SLOTS], f32,
                              kind="ExternalOutput")

    with tile.TileContext(nc) as tc, ExitStack() as ctx:
        # ================= persistent pools =================
        const_p = ctx.enter_context(tc.tile_pool(name="const", bufs=1))
        ident = const_p.tile([128, 128], f32)
        make_identity(nc, ident[:])
        triT_sb = const_p.tile([Tm1, Tm1], f32)
        nc.sync.dma_start(triT_sb[:], d_triT.ap())
        whh16_sb = const_p.tile([128, 2, 3 * HID], bf16)
        for k in range(2):
            nc.sync.dma_start(whh16_sb[:, k, :],
                              d_whh16.ap()[k * 128:(k + 1) * 128, :])
        biasc_sb = const_p.tile([128, 6], f32)
        nc.sync.dma_start(biasc_sb[:], d_biasc.ap())
        bhhn_sb = const_p.tile([128, 2], f32)
        nc.sync.dma_start(bhhn_sb[:], d_bhhn.ap())
        ws1T_sb = const_p.tile([128, 2, HID], f32)
        for k in range(2):
            nc.sync.dma_start(ws1T_sb[:, k, :], d_ws1T.ap()[k * 128:(k + 1) * 128, :])
        ws2T_sb = const_p.tile([128, 2, HEADS], f32)
        for k in range(2):
            nc.sync.dma_start(ws2T_sb[:, k, :], d_ws2T.ap()[k * 128:(k + 1) * 128, :])

        big_p = ctx.enter_context(tc.tile_pool(name="big", bufs=1))
        # gi: free layout t*48 + m*8 + b
        gi_sb = big_p.tile([128, Tm1 * 6 * BC], f32)
        # H stores per chain: [128, kappa, t-slot(Tm1+1), bch]
        Hst = [big_p.tile([128, 2, Tm1 + 1, BCH], f32, name=f"Hst{c}")
               for c in range(NCHAIN)]

        # h0 into slot 0
        for c in range(NCHAIN):
            for k in range(2):
                nc.sync.dma_start(
                    Hst[c][:, k, 0, :],
                    d_h0T.ap()[k * 128:(k + 1) * 128, c * BCH:(c + 1) * BCH])

        # ================= phase 1: gather + transpose + gi GEMM ==========
        with tc.tile_pool(name="p1w", bufs=1) as p1w, \
             tc.tile_pool(name="p1", bufs=2) as p1, \
             tc.tile_pool(name="p1ps", bufs=2, space="PSUM") as p1ps, \
             tc.tile_pool(name="xt", bufs=1) as xtp:
            # Wih^T K-tiles
            wih_k = []
            for kt in range(8):
                w = p1w.tile([128, 3 * HID], f32, name=f"wih{kt}")
                nc.sync.dma_start(
                    w[0:KTILE_ROWS[kt], :],
                    d_wihT.ap()[KTILE_WROW[kt]:KTILE_WROW[kt] + KTILE_ROWS[kt], :])
                wih_k.append(w)
            # x^T K-tiles
            xT = [xtp.tile([128, PAIRS], f32, name=f"xT{kt}") for kt in range(8)]

            for ci in range(len(chunk_bounds) - 1):
                p0, p1e = chunk_bounds[ci], chunk_bounds[ci + 1]
                np_c = p1e - p0
                stg = p1.tile([128, 1024], f32, tag="stg")
                lidx = p1.tile([128, 1], i32, tag="lidx")
                nidx = p1.tile([128, 1], i32, tag="nidx")
                bidx = p1.tile([128, 1], i32, tag="bidx")
                nc.sync.dma_start(lidx[0:np_c, :], d_lidx.ap()[p0:p1e, :])
                nc.sync.dma_start(nidx[0:np_c, :], d_nidx.ap()[p0:p1e, :])
                nc.sync.dma_start(bidx[0:np_c, :], d_bofp.ap()[p0:p1e, :])
                nc.gpsimd.indirect_dma_start(
                    out=stg[0:np_c, 0:E], out_offset=None,
                    in_=d_label.ap(),
                    in_offset=IndirectOffsetOnAxis(ap=lidx[0:np_c, 0:1], axis=0))
                nc.gpsimd.indirect_dma_start(
                    out=stg[0:np_c, 384:384 + ENC], out_offset=None,
                    in_=d_prog.ap(),
                    in_offset=IndirectOffsetOnAxis(ap=bidx[0:np_c, 0:1], axis=0))
                nc.gpsimd.indirect_dma_start(
                    out=stg[0:np_c, 640:640 + E], out_offset=None,
                    in_=d_node.ap(),
                    in_offset=IndirectOffsetOnAxis(ap=nidx[0:np_c, 0:1], axis=0))
                for kt in range(8):
                    pt = p1ps.tile([128, 128], f32, tag="tp", space="PSUM")
                    c0 = KTILE_SRC_COL[kt]
                    nc.tensor.transpose(pt[0:128, 0:np_c],
                                        stg[0:np_c, c0:c0 + 128],
                                        ident[0:np_c, 0:np_c])
                    kr = KTILE_ROWS[kt]
                    nc.vector.tensor_copy(xT[kt][0:kr, p0:p1e], pt[0:kr, 0:np_c])

            # gi GEMM: out gi^T [768, PAIRS] in m-tiles
            nhalf = (PAIRS // 2) // BC * BC  # multiple of BC
            for (n0, n1) in [(0, nhalf), (nhalf, PAIRS)]:
                nsz = n1 - n0
                for m in range(6):
                    ps = p1ps.tile([128, 512], f32, tag="gemm", space="PSUM")
                    for kt in range(8):
                        kr = KTILE_ROWS[kt]
                        nc.tensor.matmul(
                            ps[:, 0:nsz],
                            lhsT=wih_k[kt][0:kr, m * 128:(m + 1) * 128],
                            rhs=xT[kt][0:kr, n0:n1],
                            start=(kt == 0), stop=(kt == 7))
                    # bias add + scatter into gi layout [t*48 + m*8 + b]
                    t0 = n0 // BC
                    nt = nsz // BC
                    gi_view = gi_sb[:].rearrange(
                        "p (t x) -> p t x", x=6 * BC)[:, t0:t0 + nt,
                                                      m * BC:(m + 1) * BC]
                    nc.vector.tensor_scalar_add(
                        gi_view, ps[:].rearrange("p (t b) -> p t b", b=BC)[:, 0:nt, :],
                        biasc_sb[:, m:m + 1])

        # ================= phase 2: GRU loop =================
        with tc.tile_pool(name="p2ps", bufs=2, space="PSUM") as p2ps, \
             tc.tile_pool(name="p2w", bufs=1, space="PSUM") as p2wps, \
             tc.tile_pool(name="p2", bufs=2) as p2:
            gi_v = gi_sb[:].rearrange("p (t m b) -> p t m b", m=6, b=BC)
            # fp16 copy of h for the recurrent matmul (weights fp16 + FWL)
            h16 = []
            for c in range(NCHAIN):
                ht = p2.tile([128, 2, BCH], bf16, tag=f"h16_{c}",
                             name=f"h16i{c}")
                for k in range(2):
                    nc.sync.dma_start(
                        ht[:, k, :],
                        d_h016.ap()[k * 128:(k + 1) * 128,
                                    c * BCH:(c + 1) * BCH])
                h16.append(ht)
            for t in range(Tm1):
                for c in range(NCHAIN):
                    b0 = c * BCH
                    ps_rz = p2ps.tile([128, 4 * BCH], f32, tag=f"rz{c}", space="PSUM")
                    ps_hn = p2ps.tile([128, 2 * BCH], f32, tag=f"hn{c}", space="PSUM", bufs=1)
                    # preload gi (r,z part) into psum via identity matmul
                    # (exact; off the h-dependency path)
                    nc.tensor.matmul(
                        ps_rz[:].rearrange("p (m b) -> p m b", b=BCH),
                        lhsT=ident[:], rhs=gi_v[:, t, 0:4, b0:b0 + BCH],
                        start=True, stop=False)
                    for m in range(6):
                        out_ap = (ps_rz[:, m * BCH:(m + 1) * BCH] if m < 4
                                  else ps_hn[:, (m - 4) * BCH:(m - 3) * BCH])
                        for k in range(2):
                            nc.tensor.matmul(
                                out_ap,
                                lhsT=whh16_sb[:, k, m * 128:(m + 1) * 128],
                                rhs=h16[c][:, k, :],
                                start=(m == 4 and k == 0),
                                stop=(k == 1) and (m == 3 or m == 5))
                    rz = p2.tile([128, 4 * BCH], f32, tag=f"sig{c}")
                    nc.scalar.activation(rz[:], ps_rz[:], AF.Sigmoid)
                    # n-pre: (hn + b_hh_n) * r + inn, fused per kappa block
                    npre = p2.tile([128, 2 * BCH], f32, tag=f"npre{c}")
                    for k in range(2):
                        nc.vector.scalar_tensor_tensor(
                            npre[:, k * BCH:(k + 1) * BCH],
                            ps_hn[:, k * BCH:(k + 1) * BCH],
                            bhhn_sb[:, k:k + 1],
                            rz[:, k * BCH:(k + 1) * BCH],
                            op0=OP.add, op1=OP.mult)
                    nc.vector.tensor_tensor(
                        npre[:].rearrange("p (m b) -> p m b", b=BCH),
                        npre[:].rearrange("p (m b) -> p m b", b=BCH),
                        gi_v[:, t, 4:6, b0:b0 + BCH], op=OP.add)
                    nt_ = p2.tile([128, 2 * BCH], f32, tag=f"tanh{c}")
                    nc.scalar.activation(nt_[:], npre[:], AF.Tanh)
                    hmn = p2.tile([128, 2 * BCH], f32, tag=f"hmn{c}")
                    nc.vector.tensor_tensor(
                        hmn[:].rearrange("p (k b) -> p k b", b=BCH),
                        Hst[c][:, :, t, :],
                        nt_[:].rearrange("p (k b) -> p k b", b=BCH),
                        op=OP.subtract)
                    zm = p2.tile([128, 2 * BCH], f32, tag=f"zm{c}")
                    nc.vector.tensor_tensor(zm[:], rz[:, 2 * BCH:4 * BCH], hmn[:],
                                            op=OP.mult)
                    nc.vector.tensor_tensor(
                        Hst[c][:, :, t + 1, :],
                        nt_[:].rearrange("p (k b) -> p k b", b=BCH),
                        zm[:].rearrange("p (k b) -> p k b", b=BCH),
                        op=OP.add)
                    hnew = p2.tile([128, 2, BCH], bf16, tag=f"h16_{c}",
                                   name=f"h16_{c}_{t}")
                    nc.vector.tensor_copy(
                        hnew[:], Hst[c][:, :, t + 1, :])
                    h16[c] = hnew
                    # warm-keeper: junk matmul tied to a late-gate tile so the
                    # PE clock monitor sees activity during the gate phase
                    jp = p2wps.tile([128, 4 * BCH], f32, tag=f"warm{c}",
                                    space="PSUM")
                    nc.tensor.matmul(jp[:, 0:2 * BCH], lhsT=ident[:],
                                     rhs=zm[:], start=True, stop=True)

        # ================= phase 3a: scores / E =================
        p3a = ctx.enter_context(tc.tile_pool(name="p3a", bufs=1))
        E_T = []  # per chain [HEADS, Tm1*BCH]
        with tc.tile_pool(name="p3aps", bufs=2, space="PSUM") as psp:
            for c in range(NCHAIN):
                ncols = Tm1 * BCH
                a1 = p3a.tile([128, 2, ncols], f32, name=f"a1_{c}")
                for m in range(2):
                    ps = psp.tile([128, 512], f32, tag="a1ps", space="PSUM")
                    for k in range(2):
                        nc.tensor.matmul(
                            ps[:, 0:ncols],
                            lhsT=ws1T_sb[:, k, m * 128:(m + 1) * 128],
                            rhs=Hst[c][:, k, 1:Tm1 + 1, :],
                            start=(k == 0), stop=(k == 1))
                    nc.scalar.activation(a1[:, m, :], ps[:, 0:ncols], AF.Tanh)
                et = p3a.tile([HEADS, ncols], f32, name=f"eT{c}")
                ps2 = psp.tile([HEADS, 512], f32, tag="sps", space="PSUM")
                for k in range(2):
                    nc.tensor.matmul(ps2[:, 0:ncols],
                                     lhsT=ws2T_sb[:, k, :],
                                     rhs=a1[:, k, :],
                                     start=(k == 0), stop=(k == 1))
                nc.scalar.activation(et[:], ps2[:, 0:ncols], AF.Exp)
                E_T.append(et)

        # ================= phase 3b: per-sample attention =================
        with tc.tile_pool(name="p3b", bufs=2) as p3b, \
             tc.tile_pool(name="p3bps", bufs=2, space="PSUM") as p3bps, \
             tc.tile_pool(name="p3bo", bufs=2) as p3bo:
            for bl in range(BC):
                c, off = bl // BCH, bl % BCH
                # E_b [Tm1, HEADS]
                psE = p3bps.tile([Tm1, HEADS], f32, tag="psE", space="PSUM")
                eT_b = E_T[c][:].rearrange("h (t b) -> h t b", b=BCH)[:, :, off]
                nc.tensor.transpose(psE[:], eT_b, ident[0:HEADS, 0:HEADS])
                E_b = p3b.tile([Tm1, HEADS], f32, tag="Eb")
                nc.vector.tensor_copy(E_b[:], psE[:])
                # Haug [Tm1, 257]
                Haug = p3b.tile([Tm1, 2 * 128 + 1], f32, tag="Haug")
                for k in range(2):
                    psT = p3bps.tile([Tm1, 128], f32, tag="psT", space="PSUM")
                    nc.tensor.transpose(psT[:], Hst[c][:, k, 1:Tm1 + 1, off],
                                        ident[:])
                    nc.vector.tensor_copy(Haug[:, k * 128:(k + 1) * 128], psT[:])
                nc.gpsimd.memset(Haug[:, 256:257], 1.0)

                ctxb = p3bo.tile([Tm1, ATT], bf16, tag="ctxb")
                alph = p3bo.tile([Tm1, Tm1 * HEADS], f32, tag="alph")
                for h in range(HEADS):
                    At = p3b.tile([Tm1, Tm1], f32, tag="At")
                    nc.vector.tensor_scalar_mul(At[:], triT_sb[:], E_b[:, h:h + 1])
                    psN = p3bps.tile([Tm1, 257], f32, tag="psN", space="PSUM")
                    nc.tensor.matmul(psN[:], lhsT=At[:], rhs=Haug[:],
                                     start=True, stop=True)
                    rZ = p3b.tile([Tm1, 1], f32, tag="rZ")
                    nc.vector.reciprocal(rZ[:], psN[:, 256:257])
                    nc.vector.tensor_scalar_mul(
                        ctxb[:, h * HID:(h + 1) * HID], psN[:, 0:HID], rZ[:])
                    psA = p3bps.tile([Tm1, Tm1], f32, tag="psA", space="PSUM")
                    nc.tensor.transpose(psA[:], At[:], ident[0:Tm1, 0:Tm1])
                    alph_v = alph[:].rearrange("t (j h) -> t j h", h=HEADS)
                    nc.vector.tensor_scalar_mul(alph_v[:, :, h], psA[:], rZ[:])
                # write alphas rows [t, bl, :, :]
                nc.sync.dma_start(
                    d_alphas.ap()[:, bl, :, :].rearrange("t j h -> t (j h)"),
                    alph[:])
                # scatter ctx rows to slots
                sidx = p3b.tile([Tm1, 1], i32, tag="sidx")
                nc.sync.dma_start(sidx[:], d_slot.ap()[:, bl:bl + 1])
                nc.gpsimd.indirect_dma_start(
                    out=d_ctxs.ap(), out_offset=IndirectOffsetOnAxis(
                        ap=sidx[:, 0:1], axis=0),
                    in_=ctxb[:], in_offset=None)

        # ================= phase 3c: pred head =================
        with tc.tile_pool(name="p3c", bufs=4) as p3c, \
             tc.tile_pool(name="p3cw", bufs=8) as p3cw, \
             tc.tile_pool(name="p3cps", bufs=1, space="PSUM") as p3cps, \
             tc.tile_pool(name="p3co", bufs=1) as p3co:
            ctxT = p3co.tile([128, 10, SLOTS], bf16)
            for k in range(10):
                nc.sync.dma_start_transpose(
                    ctxT[:, k, :], d_ctxs.ap()[:, k * 128:(k + 1) * 128])
            ps_pred = p3cps.tile([128, 16 * NPACK], f32, space="PSUM")
            for pk in range(NPACK):
                w = p3cw.tile([128, 10, 128], bf16, tag="wpk")
                nc.sync.dma_start(w[:], d_expw.ap()[pk])
                for k in range(10):
                    nc.tensor.matmul(
                        ps_pred[:, pk * 16:(pk + 1) * 16],
                        lhsT=w[:, k, :],
                        rhs=ctxT[:, k, pk * 16:(pk + 1) * 16],
                        start=(k == 0), stop=(k == 9))
            # bias add in packed layout; host unshard extracts slots
            biasP_sb = p3co.tile([128, 16 * NPACK], f32)
            nc.sync.dma_start(biasP_sb[:], d_biasT.ap())
            predall = p3co.tile([128, 16 * NPACK], f32)
            nc.vector.tensor_tensor(predall[:], ps_pred[:], biasP_sb[:],
                                    op=OP.add)
            nc.sync.dma_start(d_predpk.ap(), predall[:])

    nc.compile()
    return nc


# ======================= host staging =======================

def _stage(inputs, Tm1):
    """Compute per-core input maps + unshard metadata (host: slicing,
    transposes, dtype casts, and integer routing metadata only)."""
    pe = np.ascontiguousarray(inputs["program_emb"], dtype=np.float32)
    h0 = np.ascontiguousarray(inputs["h0"], dtype=np.float32)
    node_emb = np.ascontiguousarray(inputs["node_emb"], dtype=np.float32)
    label_flat = np.ascontiguousarray(
        inputs["label_emb"], dtype=np.float32).reshape((N + 1) * D, E)
    Wih = np.asarray(inputs["gru_w_ih"], dtype=np.float32)
    Whh = np.asarray(inputs["gru_w_hh"], dtype=np.float32)
    bih = np.asarray(inputs["gru_b_ih"], dtype=np.float32)
    bhh = np.asarray(inputs["gru_b_hh"], dtype=np.float32)
    ws1 = np.asarray(inputs["attn_ws1"], dtype=np.float32)
    ws2 = np.asarray(inputs["attn_ws2"], dtype=np.float32)
    pred_w = np.asarray(inputs["pred_w"], dtype=np.float32)
    pred_b = np.asarray(inputs["pred_b"], dtype=np.float32)
    trace = np.asarray(inputs["execution_trace"])
    na = np.asarray(inputs["node_assignments"])

    PAIRS = Tm1 * BC
    na_pad = np.concatenate([na, np.zeros((B, 1), na.dtype)], axis=1)

    bias_comb = np.concatenate([(bih + bhh)[:512], bih[512:768]])
    wihT = np.ascontiguousarray(Wih.T)
    whhT16 = np.ascontiguousarray(Whh.T).astype(np.float16)
    ws1T = np.ascontiguousarray(ws1.T)
    ws2T = np.ascontiguousarray(ws2.T)
    triT = (np.arange(Tm1)[None, :] >= np.arange(Tm1)[:, None]).astype(np.float32)
    triT = np.ascontiguousarray(triT)
    bias_comb_sb = np.ascontiguousarray(bias_comb.reshape(6, 128).T)
    bhh_n2 = np.ascontiguousarray(bhh[512:768].reshape(2, 128).T)

    # per-core routing
    per_core = []
    for c in range(NCORES):
        bs = np.arange(c * BC, (c + 1) * BC)
        tt = np.arange(Tm1)
        # pair p = t*BC + bl
        node_i = trace[bs][:, :Tm1].T            # [Tm1, BC]
        lab = (node_i * D + na_pad[bs[None, :], node_i]).astype(np.int32)
        experts = trace[bs][:, 1:Tm1 + 1].T.astype(np.int64)   # [Tm1, BC]
        per_core.append(dict(
            label_idx=lab.reshape(-1).astype(np.int32),
            node_idx=node_i.reshape(-1).astype(np.int32),
            experts=experts.reshape(-1),
        ))

    # expert grouping (fixed S=2 slots per group)
    G_max = 0
    for pc in per_core:
        exp = pc["experts"]
        order = np.lexsort((np.arange(PAIRS), exp))
        groups = []          # list of (expert, [pair indices])
        cur_e, cur = None, []
        for p in order:
            e = exp[p]
            if e == cur_e and len(cur) < SLOT_S:
                cur.append(p)
            else:
                if cur:
                    groups.append((cur_e, cur))
                cur_e, cur = e, [p]
        if cur:
            groups.append((cur_e, cur))
        pc["groups"] = groups
        G_max = max(G_max, len(groups))
    G_PAD = -(-G_max // (MPACK * 2)) * (MPACK * 2)   # mult of 16
    SLOTS = SLOT_S * G_PAD

    in_maps, metas = [], []
    for c in range(NCORES):
        pc = per_core[c]
        groups = pc["groups"]
        slot_idx = np.zeros((Tm1, BC), np.int32)
        slot_pair = np.full(SLOTS, -1, np.int64)
        gexp = np.zeros(G_PAD, np.int64)
        for g, (e, plist) in enumerate(groups):
            gexp[g] = e
            for si, p in enumerate(plist):
                s = g * SLOT_S + si
                slot_pair[s] = p
                slot_idx[p // BC, p % BC] = s
        # M-packed weight tiles [NPACK, 128(r), 10(k), 128(m)]
        NPACK = G_PAD // MPACK
        wsel = pred_w[gexp]                      # [G_PAD, 1280, 16]
        wt = wsel.reshape(NPACK, MPACK, 10, 128, D)      # [pk, g, k, r, j]
        wt = wt.transpose(0, 3, 2, 1, 4)                 # [pk, r, k, g, j]
        exp_wt = np.ascontiguousarray(
            wt.reshape(NPACK, 128, 10, 128)).astype(np.float16)
        # packed-layout bias: rows 16g+k, col pk*16 + g*2 + s
        bsel = pred_b[gexp]                      # [G_PAD, 16]
        slot_biasT = np.zeros((128, 16 * NPACK), np.float32)
        for g_abs in range(G_PAD):
            pk, g = g_abs // MPACK, g_abs % MPACK
            for s in range(SLOT_S):
                slot_biasT[16 * g:16 * (g + 1),
                           pk * 16 + g * SLOT_S + s] = bsel[g_abs]

        bs = slice(c * BC, (c + 1) * BC)
        in_maps.append({
            "label_flat": label_flat,
            "node_emb": node_emb,
            "prog_rows": np.ascontiguousarray(pe[bs]),
            "h0T": np.ascontiguousarray(h0[bs].T),
            "h0T16": np.ascontiguousarray(h0[bs].T).astype(np.float16),
            "wihT": wihT, "whhT16": whhT16,
            "bias_comb": bias_comb_sb, "bhh_n2": bhh_n2,
            "ws1T": ws1T, "ws2T": ws2T, "triT": triT,
            "label_idx": pc["label_idx"].reshape(-1, 1),
            "node_idx": pc["node_idx"].reshape(-1, 1),
            "b_of_pair": (np.arange(PAIRS) % BC).astype(np.int32).reshape(-1, 1),
            "slot_idx": slot_idx,
            "exp_wt": exp_wt,
            "slot_biasT": slot_biasT,
        })
        metas.append(dict(slot_pair=slot_pair))
    return in_maps, metas, G_PAD


def _unshard(results, metas, Tm1):
    outs = np.zeros((Tm1, B, D), np.float32)
    alphas = np.zeros((Tm1, B, Tm1, HEADS), np.float32)
    for c in range(NCORES):
        r = results[c]
        alphas[:, c * BC:(c + 1) * BC] = r["alphas_core"]
        slot_pair = metas[c]["slot_pair"]
        pp = r["pred_packed"]
        slots = np.nonzero(slot_pair >= 0)[0]
        ps = slot_pair[slots]
        pk, rem = slots // (MPACK * SLOT_S), slots % (MPACK * SLOT_S)
        g, s = rem // SLOT_S, rem % SLOT_S
        cols = pk * 16 + g * SLOT_S + s
        rows = (g[:, None] * 16 + np.arange(D)[None, :])
        vals = pp[rows, cols[:, None]]           # [n_valid, D]
        outs[ps // BC, c * BC + (ps % BC)] = vals
    return outs, alphas


_PROG_CACHE = {}
LAST_EXEC_NS = None


def kernel(**inputs):
    import concourse.bass as bass
    import concourse.mybir as mybir
    import concourse.tile as tile
    from concourse import bacc
    from concourse.bass_utils import run_bass_kernel_spmd

    Tm1 = T - 1
    in_maps, metas, G_PAD = _stage(inputs, Tm1)
    key = (Tm1, G_PAD)
    if key not in _PROG_CACHE:
        _PROG_CACHE[key] = _build_program(Tm1, G_PAD, mybir, bass, bacc, tile)
    nc = _PROG_CACHE[key]
    res = run_bass_kernel_spmd(nc, in_maps, list(range(NCORES)))
    global LAST_EXEC_NS
    LAST_EXEC_NS = res.exec_time_ns
    return _unshard(res.results, metas, Tm1)
